# revision 1
# baseline (speedup 1.0000x reference)
"""KoLeo loss (view-expanded) on 8 Trainium2 NeuronCores.

Reference math, per view (T=4 views of X [B=8192, D=1024] fp32):
    xn  = x / ||x||                       (row L2 normalize, fp32)
    m_i = max_{j != i} <xn_i, xn_j>       (masked Gram row max)
    dist_i = ||xn_i - xn_{argmax}|| = sqrt(2 - 2 m_i)   (unit rows; the
             reference's +1e-12 eps terms are < 1e-10 relative -> ignored)
    loss = mean_views( -mean_i log(dist_i) ) = -0.5/(T*B) * sum ln(2 - 2 m_i)

Sharding: data-parallel over query rows with symmetry exploitation. Each
of the 8 cores owns B/8=1024 query rows; its input is np.roll'ed by
-c*1024 rows so the (single SPMD) program always sees its queries as rows
0..1023. Because the Gram matrix is symmetric, each core computes only
the column window [0, 5120) in rolled coordinates (its own rows plus half
the ring, rounded up to whole 1024-col panels). Every unordered pair
{r,s} is covered by at least one endpoint's window. Each core produces:
  - row maxes over its window (per query row), and
  - column maxes over its window (max over its 128-row m-blocks,
    partition dim left unreduced),
and the host combines all partial maxes (max is idempotent, so the
overlap region double-counting is harmless), then computes the final
log-mean in float64.

Per-core device pipeline, per view:
  phase 1 (normalize, row-major): 40 chunks [128,1024] f32 stream from
    DRAM (4 chunks per DMA); ScalarE Square+accum_out produces row sums
    of squares; rsqrt = exp(-0.5*ln(n2)) on ScalarE + one fp32 Newton
    step on VectorE (all ACT funcs forced into one table set); VectorE
    tensor_scalar scales rows and casts to bf16; chunks stored to a DRAM
    scratch Xn [5120,1024] bf16 (2 scratches, view parity).
  phase 2 (Gram + maxes): DMA-transpose loads build K^T panels
    [128(d), 1024(b)] bf16 plus resident Q^T [128,1024] slices; TensorE
    accumulates G blocks into PSUM [128,1024] f32 (8 K-chunks x 2 N=512
    matmuls, 4 PSUM tiles in flight); VectorE adds a -4*I mask on the
    diagonal 128-col window (panel 0 only), row-max-reduces each block
    into the row-max buffer, and elementwise-max-accumulates blocks into
    a per-panel column-max tile [128,1024].
"""

import numpy as np

_B = 8192
_T = 4
_D = 1024
_NCORES = 8

_nc_cache = {}


def _cfg(B, T, D, ncores):
    P = 128
    NQ = B // ncores              # query rows per core
    MB = NQ // P                  # m-blocks
    QCW = 1024                    # gram columns per panel (= one PSUM tile)
    NQW = -(-(NQ + B // 2) // QCW)  # panels per core (window, rounded up)
    COLS = NQW * QCW              # column window per core
    KC = D // P                   # contraction chunks
    CH = COLS // P                # row chunks normalized per view
    GRP = min(8, CH)              # chunks per scale batch
    assert COLS <= B and NQ <= QCW and CH % GRP == 0 and D % P == 0
    return P, NQ, MB, QCW, NQW, COLS, KC, CH, GRP


def _patch_act_tables():
    """Force every ACT table load onto natural_log_exp_and_others (which
    contains square+ln+exp+copy+identity) by emptying all other sets in
    the list handed to bacc's chooser. Positions are preserved so the
    emitted act_func_set_id still indexes the real act_info.json."""
    import functools

    from concourse import bacc, hw_specs

    if getattr(bacc, "_koleo_act_patch", False):
        return
    orig = hw_specs.get_activation_tables

    @functools.cache
    def patched(arch):
        tabs = orig(arch)
        keep = "natural_log_exp_and_others"
        if keep not in tabs:
            return tabs
        return {n: (fns if n == keep else set()) for n, fns in tabs.items()}

    bacc.get_activation_tables = patched
    bacc._koleo_act_patch = True


def build_nc(B=_B, T=_T, D=_D, ncores=_NCORES, enable_asserts=False, debug=False):
    import concourse.tile as tile
    from concourse import bacc, mybir

    _patch_act_tables()

    P, NQ, MB, QCW, NQW, COLS, KC, CH, GRP = _cfg(B, T, D, ncores)
    NG = CH // GRP
    MCOLS = T * MB

    f32 = mybir.dt.float32
    bf16 = mybir.dt.bfloat16
    AF = mybir.ActivationFunctionType
    ALU = mybir.AluOpType
    AX = mybir.AxisListType

    nc = bacc.Bacc(
        "TRN2",
        target_bir_lowering=False,
        debug=debug,
        enable_asserts=enable_asserts,
    )

    x = nc.dram_tensor("x", [B, T, D], f32, kind="ExternalInput").ap()
    negdiag = nc.dram_tensor("negdiag", [P, P], f32, kind="ExternalInput").ap()
    maxes = nc.dram_tensor("maxes", [P, MCOLS], f32, kind="ExternalOutput").ap()
    colmax = nc.dram_tensor(
        "colmax", [T * NQW, P, QCW], f32, kind="ExternalOutput"
    ).ap()
    xn = [nc.dram_tensor(f"xn{i}", [COLS, D], bf16).ap() for i in range(2)]

    with tile.TileContext(nc) as tc:
        with (
            tc.tile_pool(name="consts", bufs=1) as consts,
            tc.tile_pool(name="xin", bufs=3) as xin_pool,
            tc.tile_pool(name="sq", bufs=2) as sq_pool,
            tc.tile_pool(name="xnb", bufs=2) as xnb_pool,
            tc.tile_pool(name="stats", bufs=2) as stats_pool,
            tc.tile_pool(name="small", bufs=4) as small_pool,
            tc.tile_pool(name="qt", bufs=2) as qt_pool,
            tc.tile_pool(name="kt", bufs=2) as kt_pool,
            tc.tile_pool(name="cacc", bufs=3) as cacc_pool,
            tc.tile_pool(name="acc", bufs=1) as acc_pool,
            tc.tile_pool(name="ps", bufs=4, space="PSUM") as ps_pool,
        ):
            negd = consts.tile([P, P], f32)
            nc.sync.dma_start(out=negd, in_=negdiag)

            mbuf = acc_pool.tile([P, MCOLS], f32)

            for t in range(T):
                xnt = xn[t % 2]

                # ---- phase 1: normalize rows [0, COLS), store bf16 ----
                n2 = stats_pool.tile([P, CH], f32, name=f"n2_{t}", tag="n2")
                sc = stats_pool.tile([P, CH], f32, name=f"sc_{t}", tag="sc")
                MEGA = min(4, GRP)  # row-chunks per DMA transfer
                for g in range(NG):
                    megas = []
                    for mg in range(GRP // MEGA):
                        bc0 = g * GRP + mg * MEGA
                        xm = xin_pool.tile(
                            [P, MEGA, D], f32, name=f"xin_{t}_{bc0}", tag="xin"
                        )
                        nc.sync.dma_start(
                            out=xm,
                            in_=x[bc0 * P:(bc0 + MEGA) * P, t, :].rearrange(
                                "(c p) d -> p c d", p=P
                            ),
                        )
                        for j in range(MEGA):
                            bc = bc0 + j
                            sqt = sq_pool.tile(
                                [P, D], f32, name=f"sq_{t}_{bc}", tag="sq"
                            )
                            nc.scalar.activation(
                                out=sqt,
                                in_=xm[:, j, :],
                                func=AF.Square,
                                accum_out=n2[:, bc:bc + 1],
                            )
                        megas.append((bc0, xm))

                    gs = slice(g * GRP, (g + 1) * GRP)
                    # rsqrt seed via exp(-0.5 ln(n2)) (same ACT table set as
                    # Square), then one fp32 Newton step:
                    #   s = s0 * (1.5 - 0.5 * n2 * s0^2)
                    lnv = small_pool.tile([P, GRP], f32, name=f"lnv_{t}_{g}", tag="lnv")
                    nc.scalar.activation(out=lnv, in_=n2[:, gs], func=AF.Ln)
                    s0 = small_pool.tile([P, GRP], f32, name=f"s0_{t}_{g}", tag="s0")
                    nc.scalar.activation(out=s0, in_=lnv, func=AF.Exp, scale=-0.5)
                    t1 = small_pool.tile([P, GRP], f32, name=f"t1_{t}_{g}", tag="t1")
                    nc.vector.tensor_mul(t1, s0, s0)
                    t2 = small_pool.tile([P, GRP], f32, name=f"t2_{t}_{g}", tag="t2")
                    nc.vector.tensor_mul(t2, t1, n2[:, gs])
                    t3 = small_pool.tile([P, GRP], f32, name=f"t3_{t}_{g}", tag="t3")
                    nc.vector.tensor_scalar(t3, t2, -0.5, 1.5, ALU.mult, ALU.add)
                    nc.vector.tensor_mul(sc[:, gs], s0, t3)

                    for bc0, xm in megas:
                        xnb = xnb_pool.tile(
                            [P, MEGA, D], bf16, name=f"xnb_{t}_{bc0}", tag="xnb"
                        )
                        for j in range(MEGA):
                            bc = bc0 + j
                            nc.vector.tensor_scalar_mul(
                                xnb[:, j, :], xm[:, j, :], sc[:, bc:bc + 1]
                            )
                        nc.sync.dma_start(
                            out=xnt[bc0 * P:(bc0 + MEGA) * P, :].rearrange(
                                "(c p) d -> p c d", p=P
                            ),
                            in_=xnb,
                        )

                # ---- phase 2: Gram blocks, row + column maxes ----
                qts = []
                for k in range(KC):
                    qt_t = qt_pool.tile([P, NQ], bf16, name=f"qt_{t}_{k}", tag=f"qt{k}")
                    nc.sync.dma_start_transpose(
                        out=qt_t, in_=xnt[0:NQ, k * P:(k + 1) * P]
                    )
                    qts.append(qt_t)
                for q in range(NQW):
                    kts = []
                    for k in range(KC):
                        kt_t = kt_pool.tile(
                            [P, QCW], bf16, name=f"kt_{t}_{q}_{k}", tag=f"kt{k}"
                        )
                        nc.sync.dma_start_transpose(
                            out=kt_t,
                            in_=xnt[q * QCW:(q + 1) * QCW, k * P:(k + 1) * P],
                        )
                        kts.append(kt_t)
                    cm = cacc_pool.tile([P, QCW], f32, name=f"cm_{t}_{q}", tag="cm")
                    for mi in range(MB):
                        ps = ps_pool.tile(
                            [P, QCW], f32, name=f"ps_{t}_{q}_{mi}", tag="ps"
                        )
                        for k in range(KC):
                            for nb in range(QCW // 512):
                                nc.tensor.matmul(
                                    ps[:, nb * 512:(nb + 1) * 512],
                                    qts[k][:, mi * P:(mi + 1) * P],
                                    kts[k][:, nb * 512:(nb + 1) * 512],
                                    start=(k == 0),
                                    stop=(k == KC - 1),
                                )
                        if q == 0:
                            # mask the self-dot: psum diag window += -4*I
                            nc.vector.tensor_tensor(
                                ps[:, mi * P:(mi + 1) * P],
                                ps[:, mi * P:(mi + 1) * P],
                                negd,
                                op=ALU.add,
                            )
                        col = t * MB + mi
                        if q == 0:
                            nc.vector.reduce_max(
                                mbuf[:, col:col + 1], ps, axis=AX.X
                            )
                        else:
                            qm = small_pool.tile(
                                [P, 1], f32, name=f"qm_{t}_{q}_{mi}", tag="qm"
                            )
                            nc.vector.reduce_max(qm, ps, axis=AX.X)
                            nc.vector.tensor_tensor(
                                mbuf[:, col:col + 1],
                                mbuf[:, col:col + 1],
                                qm,
                                op=ALU.max,
                            )
                        if mi == 0:
                            nc.vector.tensor_copy(cm, ps)
                        else:
                            nc.vector.tensor_tensor(cm, cm, ps, op=ALU.max)
                    nc.sync.dma_start(out=colmax[t * NQW + q, :, :], in_=cm)

            nc.sync.dma_start(out=maxes, in_=mbuf)

    nc.compile()
    return nc


def make_negdiag(maskval=-4.0):
    return (maskval * np.eye(128)).astype(np.float32)


def make_in_maps(x, B=_B, T=_T, D=_D, ncores=_NCORES):
    """x: [B, T, D] fp32 full input -> per-core rolled input maps."""
    x = np.ascontiguousarray(x, dtype=np.float32)
    assert x.shape == (B, T, D)
    nd = make_negdiag()
    NQ = B // ncores
    in_maps = []
    for c in range(ncores):
        xr = np.roll(x, -c * NQ, axis=0) if c else x
        in_maps.append({"x": np.ascontiguousarray(xr), "negdiag": nd})
    return in_maps


def combine_maxes(results, B=_B, T=_T, D=_D, ncores=_NCORES):
    """Combine per-core row/column max partials -> M [T, B] (fp64)."""
    P, NQ, MB, QCW, NQW, COLS, KC, CH, GRP = _cfg(B, T, D, ncores)
    M = np.full((T, B), -np.inf)
    for c, r in enumerate(results):
        rowmax = np.asarray(r["maxes"], dtype=np.float64)  # [128, T*MB]
        for t in range(T):
            for mi in range(MB):
                rows = (c * NQ + mi * P + np.arange(P)) % B
                M[t, rows] = np.maximum(M[t, rows], rowmax[:, t * MB + mi])
        cmx = np.asarray(r["colmax"], dtype=np.float64)  # [T*NQW, 128, QCW]
        cmx = cmx.reshape(T, NQW, P, QCW).max(axis=2).reshape(T, COLS)
        gcols = (c * NQ + np.arange(COLS)) % B
        for t in range(T):
            np.maximum.at(M[t], gcols, cmx[t])
    return M


def assemble_output(results, B=_B, T=_T, D=_D, ncores=_NCORES):
    M = combine_maxes(results, B, T, D, ncores)
    loss = -0.5 * np.log(2.0 - 2.0 * M).mean()
    return np.asarray(loss, dtype=np.float32)


def kernel(episodes_vectors: np.ndarray) -> np.ndarray:
    from concourse.bass_utils import run_bass_kernel_spmd

    key = (_B, _T, _D, _NCORES)
    if key not in _nc_cache:
        _nc_cache[key] = build_nc()
    nc = _nc_cache[key]

    in_maps = make_in_maps(episodes_vectors)
    last_err = None
    for _attempt in range(3):
        try:
            res = run_bass_kernel_spmd(nc, in_maps, list(range(_NCORES)))
            return assemble_output(res.results)
        except Exception as e:  # transient PJRT/tunnel INTERNAL errors
            last_err = e
    raise last_err


if __name__ == "__main__":
    inputs = {
        "episodes_vectors": np.random.default_rng(0)
        .standard_normal((_B, _T, _D))
        .astype(np.float32)
    }
    print(kernel(**inputs))



# revision 5
# speedup vs baseline: 1.9580x; 1.9580x over previous
"""KoLeo loss (view-expanded) on 8 Trainium2 NeuronCores.

Reference math, per view (T=4 views of X [B=8192, D=1024] fp32):
    xn  = x / ||x||                       (row L2 normalize, fp32)
    m_i = max_{j != i} <xn_i, xn_j>       (masked Gram row max)
    dist_i = ||xn_i - xn_{argmax}|| = sqrt(2 - 2 m_i)   (unit rows; the
             reference's +1e-12 eps terms are < 1e-10 relative -> ignored)
    loss = mean_views( -mean_i log(dist_i) ) = -0.5/(T*B) * sum ln(2 - 2 m_i)

Sharding: data-parallel over query rows with symmetry exploitation. Each
of the 8 cores owns B/8=1024 query rows; its input is np.roll'ed by
-c*1024 rows so the (single SPMD) program always sees its queries as rows
0..1023. Because the Gram matrix is symmetric, each core computes only
the column window [0, 5120) in rolled coordinates (its own rows plus half
the ring, rounded up to whole 1024-col panels). Every unordered pair
{r,s} is covered by at least one endpoint's window. Each core produces:
  - row maxes over its window (per query row), and
  - column maxes over its window (max over its 128-row m-blocks,
    partition dim left unreduced),
and the host combines all partial maxes (max is idempotent, so the
overlap region double-counting is harmless), then computes the final
log-mean in float64.

Per-core device pipeline (v2: fully SBUF-resident, no DRAM scratch):
  prep (per view, per 1024-row panel): [128,4,1024] f32 chunks stream
    from DRAM; ScalarE Square+accum_out produces row sums of squares;
    rsqrt = exp(-0.5*ln(n2)) on ScalarE + one fp32 Newton step on VectorE;
    VectorE tensor_scalar scales rows to bf16; TensorE transposes each
    [128,128] block into PSUM (identity moving operand); ScalarE
    evacuates PSUM -> the persistent XnT tile [128, 8(k), 5120] bf16.
  gram (per view, per panel): TensorE accumulates G blocks [128,1024]
    f32 in PSUM from the resident Q^T [128,8,1024] copy (stationary) and
    XnT (moving); VectorE masks the diagonal (panel 0), row-max-reduces
    each block, and max-accumulates the per-panel column-max tile.
  Emission interleaves gram(t, panel p) with prep(t+1, panel p) so the
  Tile scheduler overlaps next-view normalization under current-view
  Gram; a separate Q^T tile confines the cross-view WAR hazard to one
  cheap copy.
"""

import numpy as np

_B = 8192
_T = 4
_D = 1024
_NCORES = 8

_nc_cache = {}


def _cfg(B, T, D, ncores):
    P = 128
    NQ = B // ncores              # query rows per core
    MB = NQ // P                  # m-blocks
    QCW = 1024                    # gram columns per panel
    NQW = -(-(NQ + B // 2) // QCW)  # panels per core (window, rounded up)
    COLS = NQW * QCW              # column window per core
    KC = D // P                   # contraction chunks
    CH = COLS // P                # row chunks normalized per view
    GRP = CH // NQW               # chunks per panel (= rsqrt batch)
    assert COLS <= B and NQ <= QCW and D % P == 0 and GRP == 8
    return P, NQ, MB, QCW, NQW, COLS, KC, CH, GRP


def _patch_act_tables():
    """Force every ACT table load onto natural_log_exp_and_others (which
    contains square+ln+exp+copy+identity) by emptying all other sets in
    the list handed to bacc's chooser. Positions are preserved so the
    emitted act_func_set_id still indexes the real act_info.json."""
    import functools

    from concourse import bacc, hw_specs

    if getattr(bacc, "_koleo_act_patch", False):
        return
    orig = hw_specs.get_activation_tables

    @functools.cache
    def patched(arch):
        tabs = orig(arch)
        keep = "natural_log_exp_and_others"
        if keep not in tabs:
            return tabs
        return {n: (fns if n == keep else set()) for n, fns in tabs.items()}

    bacc.get_activation_tables = patched
    bacc._koleo_act_patch = True


def build_nc(
    B=_B,
    T=_T,
    D=_D,
    ncores=_NCORES,
    enable_asserts=False,
    debug=False,
    repeat=1,
):
    """repeat>1 re-emits the whole per-core program back-to-back; used only
    by test.py to measure marginal exec time above dispatch noise."""
    import concourse.tile as tile
    from concourse import bacc, mybir

    _patch_act_tables()

    P, NQ, MB, QCW, NQW, COLS, KC, CH, GRP = _cfg(B, T, D, ncores)
    MCOLS = T * MB

    f32 = mybir.dt.float32
    bf16 = mybir.dt.bfloat16
    AF = mybir.ActivationFunctionType
    ALU = mybir.AluOpType
    AX = mybir.AxisListType

    nc = bacc.Bacc(
        "TRN2",
        target_bir_lowering=False,
        debug=debug,
        enable_asserts=enable_asserts,
    )

    x = nc.dram_tensor("x", [B, T, D], f32, kind="ExternalInput").ap()
    negdiag = nc.dram_tensor("negdiag", [P, P], f32, kind="ExternalInput").ap()
    identin = nc.dram_tensor("ident", [P, P], bf16, kind="ExternalInput").ap()
    maxes = nc.dram_tensor("maxes", [P, MCOLS], f32, kind="ExternalOutput").ap()
    colmax = nc.dram_tensor(
        "colmax", [T * NQW, P, QCW], f32, kind="ExternalOutput"
    ).ap()

    with tile.TileContext(nc) as tc:
        with (
            tc.tile_pool(name="consts", bufs=1) as consts,
            tc.tile_pool(name="xnt", bufs=1) as xnt_pool,
            tc.tile_pool(name="qt", bufs=2) as qt_pool,
            tc.tile_pool(name="xin", bufs=3) as xin_pool,
            tc.tile_pool(name="sq", bufs=2) as sq_pool,
            tc.tile_pool(name="stats", bufs=2) as stats_pool,
            tc.tile_pool(name="small", bufs=4) as small_pool,
            tc.tile_pool(name="xnb", bufs=2) as xnb_pool,
            tc.tile_pool(name="cm", bufs=2) as cm_pool,
            tc.tile_pool(name="acc", bufs=1) as acc_pool,
            tc.tile_pool(name="tp", bufs=2, space="PSUM") as tp_pool,
            tc.tile_pool(name="ps", bufs=3, space="PSUM") as ps_pool,
        ):
            negd = consts.tile([P, P], f32)
            nc.sync.dma_start(out=negd, in_=negdiag)
            idt = consts.tile([P, P], bf16)
            nc.sync.dma_start(out=idt, in_=identin)

            mbuf = acc_pool.tile([P, MCOLS], f32)
            # persistent transposed-normalized window [d-chunk k][cols]
            xnt = xnt_pool.tile([P, KC, COLS], bf16)

            stats = {}

            def get_stats(t):
                if t not in stats:
                    n2 = stats_pool.tile([P, CH], f32, name=f"n2_{t}", tag="n2")
                    sc = stats_pool.tile([P, CH], f32, name=f"sc_{t}", tag="sc")
                    stats[t] = (n2, sc)
                return stats[t]

            def emit_prep_panel(tt, p):
                rep, t = divmod(tt, T)
                n2, sc = get_stats(tt)
                halves = []
                for half in range(2):
                    c0 = p * GRP + half * 4
                    xm = xin_pool.tile(
                        [P, 4, D], f32, name=f"xin_{tt}_{c0}", tag="xin"
                    )
                    nc.sync.dma_start(
                        out=xm,
                        in_=x[c0 * P:(c0 + 4) * P, t, :].rearrange(
                            "(c p) d -> p c d", p=P
                        ),
                    )
                    for j in range(4):
                        c = c0 + j
                        sqt = sq_pool.tile([P, D], bf16, name=f"sq_{tt}_{c}", tag="sq")
                        nc.scalar.activation(
                            out=sqt,
                            in_=xm[:, j, :],
                            func=AF.Square,
                            accum_out=n2[:, c:c + 1],
                        )
                    halves.append((c0, xm))

                # rsqrt seed via exp(-0.5 ln(n2)) (same ACT table set as
                # Square), then one fp32 Newton step:
                #   s = s0 * (1.5 - 0.5 * n2 * s0^2)
                gs = slice(p * GRP, (p + 1) * GRP)
                lnv = small_pool.tile([P, GRP], f32, name=f"lnv_{tt}_{p}", tag="lnv")
                nc.scalar.activation(out=lnv, in_=n2[:, gs], func=AF.Ln)
                s0 = small_pool.tile([P, GRP], f32, name=f"s0_{tt}_{p}", tag="s0")
                nc.scalar.activation(out=s0, in_=lnv, func=AF.Exp, scale=-0.5)
                t1 = small_pool.tile([P, GRP], f32, name=f"t1_{tt}_{p}", tag="t1")
                nc.vector.tensor_mul(t1, s0, s0)
                t2 = small_pool.tile([P, GRP], f32, name=f"t2_{tt}_{p}", tag="t2")
                nc.vector.tensor_mul(t2, t1, n2[:, gs])
                t3 = small_pool.tile([P, GRP], f32, name=f"t3_{tt}_{p}", tag="t3")
                nc.vector.tensor_scalar(t3, t2, -0.5, 1.5, ALU.mult, ALU.add)
                nc.vector.tensor_mul(sc[:, gs], s0, t3)

                for c0, xm in halves:
                    for j in range(4):
                        c = c0 + j
                        xnb = xnb_pool.tile(
                            [P, D], bf16, name=f"xnb_{tt}_{c}", tag="xnb"
                        )
                        nc.vector.tensor_scalar_mul(
                            xnb, xm[:, j, :], sc[:, c:c + 1]
                        )
                        tp = tp_pool.tile(
                            [P, KC, P], bf16, name=f"tp_{tt}_{c}", tag="tp"
                        )
                        for k in range(KC):
                            nc.tensor.transpose(
                                tp[:, k, :], xnb[:, k * P:(k + 1) * P], idt
                            )
                        nc.scalar.activation(
                            out=xnt[:, :, c * P:(c + 1) * P], in_=tp, func=AF.Copy
                        )

            def emit_gram_panel(tt, p, qt):
                rep, t = divmod(tt, T)
                cm = cm_pool.tile([P, QCW], f32, name=f"cm_{tt}_{p}", tag="cm")
                for mi in range(MB):
                    ps = ps_pool.tile(
                        [P, QCW], f32, name=f"ps_{tt}_{p}_{mi}", tag="ps"
                    )
                    for k in range(KC):
                        for nb in range(QCW // 512):
                            nc.tensor.matmul(
                                ps[:, nb * 512:(nb + 1) * 512],
                                qt[:, k, mi * P:(mi + 1) * P],
                                xnt[:, k, p * QCW + nb * 512:p * QCW + (nb + 1) * 512],
                                start=(k == 0),
                                stop=(k == KC - 1),
                            )
                    if p == 0:
                        # mask the self-dot: psum diag window += -4*I
                        nc.vector.tensor_tensor(
                            ps[:, mi * P:(mi + 1) * P],
                            ps[:, mi * P:(mi + 1) * P],
                            negd,
                            op=ALU.add,
                        )
                    col = t * MB + mi
                    if p == 0:
                        nc.vector.reduce_max(mbuf[:, col:col + 1], ps, axis=AX.X)
                    else:
                        qm = small_pool.tile(
                            [P, 1], f32, name=f"qm_{tt}_{p}_{mi}", tag="qm"
                        )
                        nc.vector.reduce_max(qm, ps, axis=AX.X)
                        nc.vector.tensor_tensor(
                            mbuf[:, col:col + 1],
                            mbuf[:, col:col + 1],
                            qm,
                            op=ALU.max,
                        )
                    if mi == 0:
                        nc.vector.tensor_copy(cm, ps)
                    else:
                        nc.vector.tensor_tensor(cm, cm, ps, op=ALU.max)
                nc.sync.dma_start(out=colmax[t * NQW + p, :, :], in_=cm)

            # ---- emission schedule: pipeline prep(t+1) under gram(t) ----
            TT = repeat * T
            for p in range(NQW):
                emit_prep_panel(0, p)
            for tt in range(TT):
                qt = qt_pool.tile([P, KC, NQ], bf16, name=f"qt_{tt}", tag="qt")
                nc.vector.tensor_copy(qt, xnt[:, :, 0:NQ])
                for p in range(NQW):
                    emit_gram_panel(tt, p, qt)
                    if tt + 1 < TT:
                        emit_prep_panel(tt + 1, p)

            nc.sync.dma_start(out=maxes, in_=mbuf)

    nc.compile()
    return nc


def make_negdiag(maskval=-4.0):
    return (maskval * np.eye(128)).astype(np.float32)


def make_ident():
    from concourse import mybir

    return np.eye(128).astype(mybir.dt.np(mybir.dt.bfloat16))


def make_in_maps(x, B=_B, T=_T, D=_D, ncores=_NCORES):
    """x: [B, T, D] fp32 full input -> per-core rolled input maps."""
    x = np.ascontiguousarray(x, dtype=np.float32)
    assert x.shape == (B, T, D)
    nd = make_negdiag()
    idt = make_ident()
    NQ = B // ncores
    in_maps = []
    for c in range(ncores):
        xr = np.roll(x, -c * NQ, axis=0) if c else x
        in_maps.append(
            {"x": np.ascontiguousarray(xr), "negdiag": nd, "ident": idt}
        )
    return in_maps


def combine_maxes(results, B=_B, T=_T, D=_D, ncores=_NCORES):
    """Combine per-core row/column max partials -> M [T, B] (fp64)."""
    P, NQ, MB, QCW, NQW, COLS, KC, CH, GRP = _cfg(B, T, D, ncores)
    M = np.full((T, B), -np.inf)
    for c, r in enumerate(results):
        rowmax = np.asarray(r["maxes"], dtype=np.float64)  # [128, T*MB]
        for t in range(T):
            for mi in range(MB):
                rows = (c * NQ + mi * P + np.arange(P)) % B
                M[t, rows] = np.maximum(M[t, rows], rowmax[:, t * MB + mi])
        cmx = np.asarray(r["colmax"], dtype=np.float64)  # [T*NQW, 128, QCW]
        cmx = cmx.reshape(T, NQW, P, QCW).max(axis=2).reshape(T, COLS)
        gcols = (c * NQ + np.arange(COLS)) % B
        for t in range(T):
            np.maximum.at(M[t], gcols, cmx[t])
    return M


def assemble_output(results, B=_B, T=_T, D=_D, ncores=_NCORES):
    M = combine_maxes(results, B, T, D, ncores)
    loss = -0.5 * np.log(2.0 - 2.0 * M).mean()
    return np.asarray(loss, dtype=np.float32)


def kernel(episodes_vectors: np.ndarray) -> np.ndarray:
    from concourse.bass_utils import run_bass_kernel_spmd

    key = (_B, _T, _D, _NCORES)
    if key not in _nc_cache:
        _nc_cache[key] = build_nc()
    nc = _nc_cache[key]

    in_maps = make_in_maps(episodes_vectors)
    last_err = None
    for _attempt in range(3):
        try:
            res = run_bass_kernel_spmd(nc, in_maps, list(range(_NCORES)))
            return assemble_output(res.results)
        except Exception as e:  # transient PJRT/tunnel INTERNAL errors
            last_err = e
    raise last_err


if __name__ == "__main__":
    inputs = {
        "episodes_vectors": np.random.default_rng(0)
        .standard_normal((_B, _T, _D))
        .astype(np.float32)
    }
    print(kernel(**inputs))
